# revision 33
# baseline (speedup 1.0000x reference)
"""TRN2 Bass kernel for nn_LoRACuetLinear (equivariant LoRA linear).

Math: for each irrep block j (9 blocks of 192 features; block j uses irrep
k(j) in {0,1,2}), out_seg = seg @ W_eff[k] where
  W_eff[k] = pw_base * Wb[k] + SCALING * pw_base * pw_B * (WA[k] @ WB[k])
(the LoRA branch folds exactly into the base weight since everything is
linear).

Device strategy (8 cores, data-parallel over nodes):
  - Host transposes x to x_T [1792(pad), rows] per core so the contraction
    dim (mul/feature) lies on SBUF partitions; the device then runs
    weights-stationary matmuls out_T = W^T x_T with the moving dim = rows.
  - Default mode "f16x1": single-pass fp16.  x, W and the output all travel
    as fp16; products accumulate in fp32 PSUM.  Measured absmax_rel ~5e-4,
    well inside the 2e-2 gate, for 1/3 the PE work and 1/2 the DMA bytes of
    the 3-pass scheme.  fp16 matmuls run 1 cyc/row on the PE with separate,
    overlappable LDWEIGHTS and keep the HAM clock at 2.4 GHz.
  - Mode "f16x3" (fallback, ~3e-7): splits x and W into fp16 high/low pairs
    and accumulates x1@w1 + x2@w1 + x1@w2.
  - Weights are packed per 128-row output section into a block-diagonal
    [128, 32*128] layout so every matmul has M=128 at psum partition base 0
    (fp32-family matmuls cannot target high PE column groups on TRN2, and
    this also keeps all DMA transfers 128-partition aligned).
  - The pad region (feature rows 1728..1791 = chunk 13, partitions 64..127)
    carries no data: its input DMA, psum->sbuf copy and output DMA are
    skipped by splitting each transfer into a full chunks-0..12 piece and a
    64-partition chunk-13 piece.
  - psum->sbuf copies run on the Scalar engine; host un-transposes the
    gathered per-core outputs.
"""

import sys

sys.path.insert(0, "/opt/trn_rl_repo")

import os
import numpy as np

import concourse.bass as bass
import concourse.tile as tile
from concourse import bacc, mybir
from concourse.bass_utils import run_bass_kernel_spmd

# ---- problem constants (hardcoded per contract) ----
MUL = 192
DIMS = (1, 3, 5)
RANK = 8
SCALING = 2.0
N_NODES = 50000
FEAT = MUL * sum(DIMS)  # 1728
NCORES = 8
ROWS = N_NODES // NCORES  # 6250
FPAD = 1792  # 14 * 128
NSEC = FPAD // 128  # 14
RF16 = 512  # row-tile (moving dim); psum bank holds 512 fp32 per partition
MODE = os.environ.get("LORA_KERNEL_MODE", "f16x1")  # f16x1 | f16x3
# Filler matmuls per row-tile (f16x1): the PE HAM clock-gate drops the PE to
# 1.2 GHz whenever the PE idles through its ~3.4us activity window.  At the
# ridge (PE-warm ~8.1us/tile < DMA ~10us/tile) the PE would idle every tile
# and flap between 1.2/2.4 GHz.  Dummy matmuls into a scratch psum bank (no
# DMA cost) pin PE busy ~100% so the clock stays at 2.4 GHz and the span
# stays DMA-bound.
NDUMMY = int(os.environ.get("LORA_DUMMY_MMS", "9"))
# Garbage matmuls issued at t=0 (no data dependencies) so the PE is busy from
# the first microsecond: the HAM unthrottle takes ~20us of dense activity, so
# the clock is warming while the DMA pipeline fills instead of after.
NWARMUP = int(os.environ.get("LORA_WARMUP_MMS", "30"))
BLK_IRREP = [0] + [1] * 3 + [2] * 5


def _section_mms():
    """Enumerate matmuls as (section, chunk, r0, r1, windex).

    Section s covers padded output rows [128s, 128s+128); chunk c covers
    padded input rows [128c, 128c+128).  (s, c) participates iff the
    block-diagonal weight has support there; r0:r1 is the nonzero input-row
    range within the chunk (always base 0 or 64, size 64 or 128).
    """
    sup = np.zeros((FPAD, FPAD), dtype=bool)
    for j in range(sum(DIMS)):
        sup[192 * j : 192 * j + 192, 192 * j : 192 * j + 192] = True
    mms = []
    wi = 0
    for s in range(NSEC):
        for c in range(NSEC):
            sl = sup[128 * c : 128 * c + 128, 128 * s : 128 * s + 128]
            nz = np.nonzero(sl.any(axis=1))[0]
            if len(nz) == 0:
                continue
            r0 = (int(nz[0]) // 64) * 64
            r1 = ((int(nz[-1]) + 64) // 64) * 64
            mms.append((s, c, r0, r1, wi))
            wi += 1
    return mms


_MMS = _section_mms()
NW = len(_MMS)  # 32 packed weight slots of [128, 128]


def _pack_weights(W_eff):
    """Build the packed per-section weight [128, NW*128] from W_eff [3,192,192]."""
    W_big = np.zeros((FPAD, FPAD), dtype=np.float32)
    for j, k in enumerate(BLK_IRREP):
        W_big[192 * j : 192 * j + 192, 192 * j : 192 * j + 192] = W_eff[k]
    wpk = np.zeros((128, NW * 128), dtype=np.float32)
    for s, c, r0, r1, wi in _MMS:
        wpk[:, wi * 128 : (wi + 1) * 128] = W_big[
            128 * c : 128 * c + 128, 128 * s : 128 * s + 128
        ]
    return wpk


def _row_tiles():
    """Row-tile schedule: two small tiles first so the first matmuls start
    after ~0.45MB of DMA while the queues spin up, then full 512-row tiles,
    plus the ragged 106-row tail (6250 = 128 + 384 + 11*512 + 106).  The
    small tail keeps the final output DMA - the only non-overlapped one -
    short."""
    rows = [128, 384] + [512] * 11 + [106]
    tiles = []
    r0 = 0
    for rt in rows:
        tiles.append((r0, rt))
        r0 += rt
    assert r0 == ROWS
    return tiles


def _build_nc(mode):
    f32 = mybir.dt.float32
    f16 = mybir.dt.float16
    three_pass = mode == "f16x3"

    nc = bacc.Bacc("TRN2", target_bir_lowering=False, debug=False)
    # host pre-tiles x as [rowtile, partition, chunk*R] so each partition's
    # per-rowtile data is one contiguous segment for the DMA
    nt = len(_row_tiles())
    x1_in = nc.declare_dram_parameter("x1", [nt, 128, NSEC * RF16], f16, isOutput=False)
    if three_pass:
        x2_in = nc.declare_dram_parameter(
            "x2", [nt, 128, NSEC * RF16], f16, isOutput=False
        )
    wh_in = nc.declare_dram_parameter("wh", [128, NW * 128], f16, isOutput=False)
    if three_pass:
        wl_in = nc.declare_dram_parameter("wl", [128, NW * 128], f16, isOutput=False)
    ot_out = nc.declare_dram_parameter("ot", [nt, 128, NSEC * RF16], f16, isOutput=True)

    sec_list = [[m for m in _MMS if m[0] == s] for s in range(NSEC)]

    with tile.TileContext(nc) as tc:
        with (
            tc.tile_pool(name="wp", bufs=1) as wp,
            tc.tile_pool(name="hp", bufs=3) as hp,
            tc.tile_pool(name="lp", bufs=3) as lp,
            tc.tile_pool(name="op", bufs=3) as op,
            tc.tile_pool(name="ps", bufs=7, space="PSUM") as ps,
            tc.tile_pool(name="jk", bufs=1, space="PSUM") as jk,
        ):
            if not three_pass and NWARMUP > 0:
                # dependency-free garbage matmuls so the PE is busy from t=0
                warm = wp.tile([128, RF16], f16, tag="warm")
                nc.vector.memset(warm[:], 0.0)
                junk0 = jk.tile([128, RF16], f32, tag="jk")
                for _ in range(NWARMUP):
                    nc.tensor.matmul(
                        junk0[:], warm[:, 0:128], warm[:], start=True, stop=True
                    )

            # weight DMA split in two so the first sections' matmuls are not
            # gated on the whole 1MB weight transfer
            wh = wp.tile([128, NW * 128], f16, tag="wh")
            nc.sync.dma_start(wh[:, : 16 * 128], wh_in[:, : 16 * 128])
            nc.sync.dma_start(wh[:, 16 * 128 :], wh_in[:, 16 * 128 :])
            if three_pass:
                wl = wp.tile([128, NW * 128], f16, tag="wl")
            if three_pass:
                nc.sync.dma_start(wl[:], wl_in[:])

            nfull = NSEC - 1  # 13 full chunks; chunk 13 is half-height

            def load_x(dst_tile, src, rt, pieces=1):
                # per-tile DRAM layout is contiguous per partition: cols
                # [0, 13*rt) = full chunks, [13*rt, 14*rt) = half chunk 13
                # (partitions 0..63 only) -- line-rate descriptors at any rt.
                # Each dma_start serializes through the issuing engine's
                # HW-DGE ring (~0.35us each), so steady tiles use one big
                # piece; tile 0 splits so its first sections can start while
                # the DMA queues are still spinning up.
                bounds = [0] + [7 * (i + 1) for i in range(pieces - 1)] + [nfull]
                for a, b in zip(bounds, bounds[1:]):
                    nc.sync.dma_start(
                        dst_tile[:, a:b, :rt],
                        src[:, a * rt : b * rt].rearrange("p (c r) -> p c r", r=rt),
                    )
                nc.sync.dma_start(
                    dst_tile[0:64, nfull, :rt], src[0:64, nfull * rt : NSEC * rt]
                )

            for ti, (r0, rt) in enumerate(_row_tiles()):
                xh = hp.tile([128, NSEC, RF16], f16, tag="xh")
                load_x(xh, x1_in[ti], rt, pieces=2 if ti == 0 else 1)
                if three_pass:
                    xl = lp.tile([128, NSEC, RF16], f16, tag="xl")
                    load_x(xl, x2_in[ti], rt)

                ot = op.tile([128, NSEC, RF16], f16, tag="ot")
                for s in range(NSEC):
                    psum = ps.tile([128, RF16], f32, tag="ps")
                    # order so matmuls sharing a stationary slice are
                    # adjacent (lets walrus ldw-opt elide reloads)
                    if three_pass:
                        seq = [
                            (x, w, c, k0, k1, wi)
                            for _, c, k0, k1, wi in sec_list[s]
                            for x, w in ((xh, wh), (xl, wh))
                        ] + [
                            (xh, wl, c, k0, k1, wi)
                            for _, c, k0, k1, wi in sec_list[s]
                        ]
                    else:
                        seq = [
                            (xh, wh, c, k0, k1, wi) for _, c, k0, k1, wi in sec_list[s]
                        ]
                    for i, (xsrc, wsrc, c, k0, k1, wi) in enumerate(seq):
                        nc.tensor.matmul(
                            psum[:, :rt],
                            wsrc[k0:k1, wi * 128 : (wi + 1) * 128],
                            xsrc[k0:k1, c, :rt],
                            start=(i == 0),
                            stop=(i == len(seq) - 1),
                        )
                    # psum->sbuf copies alternate between the Scalar and DVE
                    # engines so neither becomes the per-tile long pole.
                    # Section 13 only produces data on psum partitions 0..63.
                    pl = slice(0, 64) if s == NSEC - 1 else slice(0, 128)
                    if s % 2 == 0:
                        nc.scalar.copy(ot[pl, s, :rt], psum[pl, :rt])
                    else:
                        nc.vector.tensor_copy(ot[pl, s, :rt], psum[pl, :rt])
                    if ti < 2 and not three_pass and s < 8:
                        # tile 0's input pieces trickle in while the DMA queues
                        # spin up; a filler matmul after each early section
                        # absorbs sub-us data-arrival gaps so the PE (and with
                        # it the HAM clock state) never goes idle
                        junk = jk.tile([128, RF16], f32, tag="jk")
                        nc.tensor.matmul(
                            junk[:, :rt],
                            wh[0:128, 0:128],
                            xh[0:128, 0, :rt],
                            start=True,
                            stop=True,
                        )
                if not three_pass and NDUMMY > 0:
                    # filler matmuls: keep the PE HAM-warm (see NDUMMY above);
                    # reads resident data, writes a never-read scratch bank
                    junk = jk.tile([128, RF16], f32, tag="jk")
                    nd = max(1, (NDUMMY * rt + RF16 - 1) // RF16)
                    for _ in range(nd):
                        nc.tensor.matmul(
                            junk[:, :rt],
                            wh[0:128, 0:128],
                            xh[0:128, 0, :rt],
                            start=True,
                            stop=True,
                        )
                # output ships on the Scalar engine's HW-DGE ring so the
                # input FIFO (sync ring) never stalls behind output waits.
                # Split so the first sections ship while later ones are still
                # being copied: write traffic stays interleaved with reads
                # (the end-of-kernel write-only drain runs at ~half rate).
                dst = ot_out[ti]
                nc.scalar.dma_start(
                    dst[:, : 7 * rt].rearrange("p (c r) -> p c r", r=rt),
                    ot[:, :7, :rt],
                )
                nc.scalar.dma_start(
                    dst[:, 7 * rt : nfull * rt].rearrange("p (c r) -> p c r", r=rt),
                    ot[:, 7:nfull, :rt],
                )
                nc.scalar.dma_start(
                    dst[0:64, nfull * rt : NSEC * rt], ot[0:64, nfull, :rt]
                )

    nc.finalize()
    return nc


_NC_CACHE = {}
_last_in_maps = None


def _get_nc(mode):
    if mode not in _NC_CACHE:
        _NC_CACHE[mode] = _build_nc(mode)
    return _NC_CACHE[mode]


def kernel(x, Wb, WA, WB):
    x = np.asarray(x, dtype=np.float32)
    Wb = np.asarray(Wb, dtype=np.float32)
    WA = np.asarray(WA, dtype=np.float32)
    WB = np.asarray(WB, dtype=np.float32)

    # fold LoRA into the base weight (float64 for the tiny weight math)
    pw_base = 1.0 / np.sqrt(np.float64(MUL))
    pw_B = 1.0 / np.sqrt(np.float64(RANK))
    W_eff = (
        pw_base * Wb.astype(np.float64)
        + SCALING * pw_base * pw_B * (WA.astype(np.float64) @ WB.astype(np.float64))
    ).astype(np.float32)

    wpk = _pack_weights(W_eff)
    wh = wpk.astype(np.float16)
    wl = (wpk - wh.astype(np.float32)).astype(np.float16)

    tiles = _row_tiles()
    three_pass = MODE == "f16x3"

    # per-core pre-tiled fp16 inputs
    in_maps = []
    for i in range(NCORES):
        xt = np.zeros((FPAD, ROWS), dtype=np.float32)
        xt[:FEAT] = x[i * ROWS : (i + 1) * ROWS].T
        x1p = xt.astype(np.float16)
        x1 = np.zeros((len(tiles), 128, NSEC * RF16), dtype=np.float16)
        if three_pass:
            x2p = (xt - x1p.astype(np.float32)).astype(np.float16)
            x2 = np.zeros_like(x1)
        nfull = NSEC - 1
        for ti, (r0, rt) in enumerate(tiles):
            for dstarr, srcarr in ((x1, x1p),) + (((x2, x2p),) if three_pass else ()):
                a = srcarr[:, r0 : r0 + rt].reshape(NSEC, 128, rt)
                dstarr[ti, :, : nfull * rt] = (
                    a[:nfull].transpose(1, 0, 2).reshape(128, nfull * rt)
                )
                dstarr[ti, :64, nfull * rt : NSEC * rt] = a[nfull, :64]
        m = {"x1": x1, "wh": wh}
        if three_pass:
            m["x2"] = x2
            m["wl"] = wl
        in_maps.append(m)

    global _last_in_maps
    _last_in_maps = in_maps
    nc = _get_nc(MODE)
    res = run_bass_kernel_spmd(nc, in_maps, core_ids=list(range(NCORES)))

    out = np.empty((N_NODES, FEAT), dtype=np.float32)
    nfull = NSEC - 1
    for i in range(NCORES):
        ot = res.results[i]["ot"]
        for ti, (r0, rt) in enumerate(tiles):
            o = ot[ti]
            full = (
                o[:, : nfull * rt]
                .reshape(128, nfull, rt)
                .transpose(1, 0, 2)
                .reshape(nfull * 128, rt)
                .astype(np.float32)
            )
            half = o[:64, nfull * rt : NSEC * rt].astype(np.float32)
            rows = slice(i * ROWS + r0, i * ROWS + r0 + rt)
            out[rows, : nfull * 128] = full.T
            out[rows, nfull * 128 :] = half.T
    return out
